# revision 26
# baseline (speedup 1.0000x reference)
"""Multi-head dilated sliding-window attention (window=129, dil=1) on 8 TRN2 cores.

Sharding: sequence-parallel. Each core computes 256 query rows (N=2048 / 8),
with a 64-row K/V halo on each side (zero-padded at the sequence edges).
Weights are replicated (resident in SBUF, bf16).

Band-softmax identity used (reference softmaxes the FULL row with zeros
outside the band):
    out_i = (sum_band (e^{s_ij} - 1) V_j + sum_all V_j) / (sum_band (e^{s_ij} - 1) + N)
computed per head with V_raw = x@Wv (no bias; bv is folded in after the
attention average), bk applied only to real (non-padding) K rows via an
indicator-row matmul, and the global sum_all V_j = (sum_n x_n) @ Wv computed
on-device from the (tiny) host-reduced x column-sum.

Compute dtype: bf16 operands into the PE (fp32 runs at quarter rate on TRN2),
fp32 PSUM accumulation and fp32 softmax arithmetic.

Structure: Q^T/K^T projections are computed per head-pair (db) and attention
for that pair runs immediately, pipelined one round behind the scores so the
PE never stalls on the ACT/DVE softmax chain.
"""

import numpy as np
import ml_dtypes
from contextlib import ExitStack

import concourse.bass as bass
import concourse.tile as tile
from concourse import bacc, mybir
from concourse.bass_utils import run_bass_kernel_spmd

F32 = mybir.dt.float32
BF16 = mybir.dt.bfloat16
NPBF16 = ml_dtypes.bfloat16
N, E, H, D = 2048, 1024, 16, 64
R = N // 8          # 256 query rows per core
HALO = R + 128      # 384 K/V rows per core
NQB = R // 128      # query blocks per core


def build_graph():
    nc = bacc.Bacc("TRN2", target_bir_lowering=False, debug=False, num_devices=8)

    xh_d = nc.declare_dram_parameter("xh", [HALO, E], BF16, isOutput=False)
    xvalid_d = nc.declare_dram_parameter("xvalid", [1, HALO], BF16, isOutput=False)
    wq_d = nc.declare_dram_parameter("Wq", [E, H * D], BF16, isOutput=False)
    wk_d = nc.declare_dram_parameter("Wk", [E, H * D], BF16, isOutput=False)
    wv_d = nc.declare_dram_parameter("Wv", [E, H * D], BF16, isOutput=False)
    wo_d = nc.declare_dram_parameter("Wo", [H * D, E], BF16, isOutput=False)
    bq_d = nc.declare_dram_parameter("bq_r", [128, 8], F32, isOutput=False)
    bk_d = nc.declare_dram_parameter("bk_row", [1, H * D], BF16, isOutput=False)
    bv_d = nc.declare_dram_parameter("bv_r", [128, 8], F32, isOutput=False)
    bo_d = nc.declare_dram_parameter("bo_row", [1, E], BF16, isOutput=False)
    xsum_d = nc.declare_dram_parameter("xsum_r", [128, 8], BF16, isOutput=False)
    m4_d = nc.declare_dram_parameter("mask4", [128, 512], F32, isOutput=False)
    id_d = nc.declare_dram_parameter("ident", [128, 128], BF16, isOutput=False)
    out_d = nc.declare_dram_parameter("out", [R, E], F32, isOutput=True)

    with tile.TileContext(nc) as tc, ExitStack() as ctx:
        const = ctx.enter_context(tc.tile_pool(name="const", bufs=1))
        pers = ctx.enter_context(tc.tile_pool(name="pers", bufs=1))
        epool = ctx.enter_context(tc.tile_pool(name="epool", bufs=3))
        ppool = ctx.enter_context(tc.tile_pool(name="ppool", bufs=5))
        zpool = ctx.enter_context(tc.tile_pool(name="zpool", bufs=4))
        obpool = ctx.enter_context(tc.tile_pool(name="obpool", bufs=2))
        psum = ctx.enter_context(tc.tile_pool(name="psum", bufs=8, space="PSUM"))

        def ps(shape, dt=F32):
            return psum.tile(shape, dt, tag="ps", name="pst")

        # ---- loads. DMA queues are round-robin by emission order and one
        # 256KB tile costs ~9.3us in a single queue (27 GB/s/port), so the
        # emission order below is a hand-scheduled queue assignment: each
        # queue's first transfer is the earliest-needed tile.
        xtiles = []
        for st in range(3):
            xt = const.tile([128, E], BF16, tag=f"xload{st}", name="xt")
            for hf in range(2):
                nc.sync.dma_start(xt[:, hf * 512:(hf + 1) * 512],
                                  xh_d[st * 128:(st + 1) * 128,
                                       hf * 512:(hf + 1) * 512])
            xtiles.append(xt)
        identity = const.tile([128, 128], BF16, tag="identity")
        nc.sync.dma_start(identity[:], id_d[:, :])

        # ---- PE clock warm-up: ~3.5us of dummy matmuls during the DMA
        # phase so the HAM clock gate is already at 8/8 when real work lands
        wu = const.tile([128, 128], BF16, tag="wu")
        nc.vector.memset(wu[:], 0.0)
        wups = psum.tile([128, 128], F32, tag="ps", name="wups")
        for _ in range(32):
            nc.tensor.matmul(wups[:], wu[:], wu[:], start=True, stop=True)

        def wtile(nm):
            return const.tile([128, E], BF16, tag=nm, name="wt")

        def wdma(wt, src_d, et):
            nc.sync.dma_start(wt[:], src_d[et * 128:(et + 1) * 128, :])

        xsum_sb = const.tile([128, 8], BF16, tag="xsum")
        nc.sync.dma_start(xsum_sb[:], xsum_d[:, :])
        wv_t = [wtile(f"wv{et}") for et in range(8)]
        wq_t = [wtile(f"wq{et}") for et in range(8)]
        wk_t = [wtile(f"wk{et}") for et in range(8)]
        wo_t = [wtile(f"wo{et}") for et in range(8)]
        m4 = const.tile([128, 512], F32, tag="m4")
        bq_sb = const.tile([128, 8], F32, tag="bq")
        bv_sb = const.tile([128, 8], F32, tag="bv")
        bk_sb = const.tile([1, H * D], BF16, tag="bk")
        bo_sb = const.tile([1, E], BF16, tag="bo")
        valid_sb = const.tile([1, HALO], BF16, tag="valid")
        # wave 1 tail (queues 8..15): wv0-5, wq0, wk0
        for et in range(6):
            wdma(wv_t[et], wv_d, et)
        wdma(wq_t[0], wq_d, 0)
        wdma(wk_t[0], wk_d, 0)
        # wave 2: early small consts into the short queues, wv6/7 behind
        # ident/xsum (the two shortest first-wave transfers)
        wdma(wq_t[1], wq_d, 1)
        wdma(wk_t[1], wk_d, 1)
        wdma(wq_t[2], wq_d, 2)
        wdma(wk_t[2], wk_d, 2)
        nc.sync.dma_start(m4[:], m4_d[:, :])
        nc.sync.dma_start(bq_sb[:], bq_d[:, :])
        wdma(wv_t[6], wv_d, 6)
        wdma(wv_t[7], wv_d, 7)
        nc.sync.dma_start(bk_sb[:], bk_d[:, :])
        nc.sync.dma_start(valid_sb[:], xvalid_d[:, :])
        wdma(wq_t[3], wq_d, 3)
        wdma(wk_t[3], wk_d, 3)
        wdma(wq_t[4], wq_d, 4)
        wdma(wk_t[4], wk_d, 4)
        wdma(wq_t[5], wq_d, 5)
        wdma(wk_t[5], wk_d, 5)
        # wave 3
        wdma(wq_t[6], wq_d, 6)
        wdma(wk_t[6], wk_d, 6)
        wdma(wq_t[7], wq_d, 7)
        wdma(wk_t[7], wk_d, 7)
        for et in range(8):
            wdma(wo_t[et], wo_d, et)
        nc.sync.dma_start(bv_sb[:], bv_d[:, :])
        nc.sync.dma_start(bo_sb[:], bo_d[:, :])
        ones_sb = const.tile([1, 128], BF16, tag="ones")
        nc.vector.memset(ones_sb[:], 1.0)
        biascat = const.tile([1, H, D + 1], BF16, tag="biascat")

        # ---- persistent activations ---------------------------------------
        xT = pers.tile([128, 8, HALO], BF16, tag="xT")       # [e_p, e_t, seq]
        QT = pers.tile([128, 8, R], BF16, tag="QT")          # [d_p, d_t, q]
        KT = pers.tile([128, 8, HALO], BF16, tag="KT")       # [d_p, d_t, seq]
        Vaug = pers.tile([128, 3, H, D + 1], BF16, tag="Vaug")
        Asc = pers.tile([128, NQB, H * D], BF16, tag="Asc")  # [q_p, qblk, dims]
        AT = pers.tile([128, 8, R], BF16, tag="AT")          # [d_p, d_t, q]

        # ---- transpose x to xT (PE transpose) -----------------------------
        for st in range(3):
            for et in range(8):
                tp = ps([128, 128], BF16)
                nc.tensor.transpose(tp[:], xtiles[st][:, et * 128:(et + 1) * 128],
                                    identity[:])
                nc.vector.tensor_copy(xT[:, et, st * 128:(st + 1) * 128], tp[:])

        # ---- V (natural layout, raw) + S_V = xsum @ Wv --------------------
        # st-serial so only 2-4 PSUM banks are held, letting the first
        # attention rounds overlap the later V blocks.
        svps = [ps([1, 512]) for _ in range(2)]
        for st in range(3):
            vp = [ps([128, 512]) for _ in range(2)]
            for et in range(8):
                for hf in range(2):
                    nc.tensor.matmul(vp[hf][:],
                                     xT[:, et, st * 128:(st + 1) * 128],
                                     wv_t[et][:, hf * 512:(hf + 1) * 512],
                                     start=(et == 0), stop=(et == 7))
                if st == 0:
                    for hf in range(2):
                        nc.tensor.matmul(svps[hf][:], xsum_sb[:, et:et + 1],
                                         wv_t[et][:, hf * 512:(hf + 1) * 512],
                                         start=(et == 0), stop=(et == 7))
            for hf in range(2):
                src = vp[hf][:].rearrange("p (h d) -> p h d", d=D)
                nc.scalar.copy(Vaug[:, st, hf * 8:(hf + 1) * 8, 0:D], src)
            if st == 0:
                for hf in range(2):
                    src = svps[hf][:].rearrange("p (h d) -> p h d", d=D)
                    nc.scalar.copy(biascat[:, hf * 8:(hf + 1) * 8, 0:D], src)
        nc.vector.memset(Vaug[:, :, :, D:D + 1], 1.0)
        nc.vector.memset(biascat[:, :, D:D + 1], 2048.0)

        # ---- fused projections + banded attention, one head-pair at a time
        # round r = db (one head pair, BOTH query blocks). Emission order:
        #   1. Q^T/K^T projection matmuls for db
        #   2. PV + bias matmuls and epilogue of round r-1 (p tiles ready)
        #   3. S matmuls (one [128, 512] psum per head = both qblk/cblk
        #      quadrants) + exp/-1/mask chain for round r
        # Per-head p layout: [q0c0 | q0c1 | q1c0 | q1c1], quadrant j uses
        # keys halo block (qblk+cblk) and mask m0/m1 alternating.
        prev = None  # (db, ptiles{h: pt}, {qblk: pv psum})

        def proj(db):
            qp = ps([128, R])
            for et in range(8):
                nc.tensor.matmul(qp[:], wq_t[et][:, db * 128:(db + 1) * 128],
                                 xT[:, et, 64:64 + R],
                                 start=(et == 0), stop=(et == 7))
            nc.scalar.add(QT[:, db, :], qp[:], bq_sb[:, db:db + 1])
            kp = ps([128, HALO])
            for et in range(8):
                nc.tensor.matmul(kp[:], wk_t[et][:, db * 128:(db + 1) * 128],
                                 xT[:, et, :], start=(et == 0), stop=False)
            nc.tensor.matmul(kp[:], bk_sb[0:1, db * 128:(db + 1) * 128],
                             valid_sb[0:1, :], start=False, stop=True)
            nc.scalar.copy(KT[:, db, :], kp[:])

        def pv_flush(pr):
            db, ptl = pr
            pvs = {}
            for qblk in range(NQB):
                pvs[qblk] = ps([128, 2 * (D + 1)])
            for qblk in range(NQB):
                pv = pvs[qblk]
                for i, h in enumerate((2 * db, 2 * db + 1)):
                    off = i * (D + 1)
                    for cblk in range(2):
                        quad = qblk * 2 + cblk
                        nc.tensor.matmul(pv[:, off:off + D + 1],
                                         ptl[h][:, quad * 128:(quad + 1) * 128],
                                         Vaug[:, qblk + cblk, h, :],
                                         start=(i == 0 and cblk == 0),
                                         stop=False)
            for qblk in range(NQB):
                pv = pvs[qblk]
                for i, h in enumerate((2 * db, 2 * db + 1)):
                    off = i * (D + 1)
                    nc.tensor.matmul(pv[:, off:off + D + 1], ones_sb[0:1, :],
                                     biascat[0:1, h, :], start=False,
                                     stop=(i == 1))
            for qblk in range(NQB):
                pv = pvs[qblk]
                for i, h in enumerate((2 * db, 2 * db + 1)):
                    off = i * (D + 1)
                    zinv = zpool.tile([128, 1], F32, tag="z", name="zinv")
                    nc.vector.reciprocal(zinv[:], pv[:, off + D:off + D + 1])
                    if i == 0:
                        nc.scalar.activation(Asc[:, qblk, h * D:(h + 1) * D],
                                             pv[:, off:off + D],
                                             mybir.ActivationFunctionType.Copy,
                                             scale=zinv[:])
                    else:
                        nc.vector.tensor_scalar_mul(
                            Asc[:, qblk, h * D:(h + 1) * D],
                            pv[:, off:off + D], zinv[:])

        for r in range(8 + 1):
            if r < 8:
                db = r
                proj(db)
                if prev is not None:
                    pv_flush(prev)
                ptl = {}
                for i, h in enumerate((2 * db, 2 * db + 1)):
                    rr = i * 64
                    sp = ps([128, 512])
                    for quad in range(4):
                        qblk, cblk = quad // 2, quad % 2
                        nc.tensor.matmul(
                            sp[:, quad * 128:(quad + 1) * 128],
                            KT[rr:rr + 64, db,
                               (qblk + cblk) * 128:(qblk + cblk + 1) * 128],
                            QT[rr:rr + 64, db, qblk * 128:(qblk + 1) * 128],
                            start=(quad == 0), stop=(quad == 3))
                    et_ = epool.tile([128, 512], F32, tag="e", name="et_")
                    nc.scalar.activation(et_[:], sp[:],
                                         mybir.ActivationFunctionType.Exp)
                    nc.vector.tensor_scalar_add(et_[:], et_[:], -1.0)
                    pt = ppool.tile([128, 512], BF16, tag="p", name="pt")
                    nc.vector.tensor_mul(pt[:], et_[:], m4[:])
                    ptl[h] = pt
                prev = (db, ptl)
            else:
                pv_flush(prev)

        # ---- transpose A (and add bv) for the output projection -----------
        for qblk in range(NQB):
            for at in range(8):
                tp = ps([128, 128], BF16)
                nc.tensor.transpose(tp[:], Asc[:, qblk, at * 128:(at + 1) * 128],
                                    identity[:])
                nc.scalar.add(AT[:, at, qblk * 128:(qblk + 1) * 128], tp[:],
                              bv_sb[:, at:at + 1])

        # ---- output projection: O = (A + bv) @ Wo + bo --------------------
        ops = [ps([128, 512]) for _ in range(2 * NQB)]
        for at in range(8):
            for qblk in range(NQB):
                for hf in range(2):
                    nc.tensor.matmul(ops[qblk * 2 + hf][:],
                                     AT[:, at, qblk * 128:(qblk + 1) * 128],
                                     wo_t[at][:, hf * 512:(hf + 1) * 512],
                                     start=(at == 0), stop=False)
        for qblk in range(NQB):
            for hf in range(2):
                nc.tensor.matmul(ops[qblk * 2 + hf][:], ones_sb[0:1, :],
                                 bo_sb[0:1, hf * 512:(hf + 1) * 512],
                                 start=False, stop=True)
        for qblk in range(NQB):
            ob = obpool.tile([128, E], F32, tag="ob")
            for hf in range(2):
                nc.vector.tensor_copy(ob[:, hf * 512:(hf + 1) * 512],
                                      ops[qblk * 2 + hf][:])
            nc.sync.dma_start(out_d[qblk * 128:(qblk + 1) * 128, :], ob[:])

    nc.compile()
    return nc


_NC = None


def get_nc():
    global _NC
    if _NC is None:
        _NC = build_graph()
    return _NC


def make_in_maps(x, Wq, bq, Wk, bk, Wv, bv, Wo, bo):
    f = lambda a: np.ascontiguousarray(np.asarray(a, dtype=np.float32))
    bf = lambda a: np.ascontiguousarray(
        np.asarray(a, dtype=np.float32).astype(NPBF16))
    x2 = f(x).reshape(N, E)
    ci = np.arange(128, dtype=np.float32)[:, None]  # key index c (partitions)
    qi = np.arange(128, dtype=np.float32)[None, :]  # query index q (free)
    m0 = (ci >= qi).astype(np.float32)
    m1 = (ci <= qi).astype(np.float32)
    mask4 = np.concatenate([m0, m1, m0, m1], axis=1)
    common = {
        "Wq": bf(Wq), "Wk": bf(Wk), "Wv": bf(Wv), "Wo": bf(Wo),
        "bq_r": f(bq).reshape(8, 128).T.copy(),
        "bk_row": bf(bk).reshape(1, H * D),
        "bv_r": f(bv).reshape(8, 128).T.copy(),
        "bo_row": bf(bo).reshape(1, E),
        "xsum_r": bf(x2.sum(0, dtype=np.float32)).reshape(8, 128).T.copy(),
        "mask4": np.ascontiguousarray(mask4),
        "ident": np.eye(128, dtype=np.float32).astype(NPBF16),
    }
    in_maps = []
    for c in range(8):
        r0 = c * R
        xh = np.zeros((HALO, E), NPBF16)
        valid = np.zeros((1, HALO), NPBF16)
        lo, hi = r0 - 64, r0 + R + 64
        slo, shi = max(lo, 0), min(hi, N)
        xh[slo - lo: shi - lo] = x2[slo:shi].astype(NPBF16)
        valid[0, slo - lo: shi - lo] = 1.0
        in_maps.append({**common, "xh": xh, "xvalid": valid})
    return in_maps


def kernel(x, Wq, bq, Wk, bk, Wv, bv, Wo, bo, _trace=False, _trace_kwargs=None):
    nc = get_nc()
    in_maps = make_in_maps(x, Wq, bq, Wk, bk, Wv, bv, Wo, bo)
    res = run_bass_kernel_spmd(nc, in_maps, list(range(8)), trace=_trace,
                               **(_trace_kwargs or {}))
    out = np.concatenate([res.results[c]["out"] for c in range(8)], axis=0)
    kernel.last_result = res
    return out[None].astype(np.float32)
